# revision 1
# baseline (speedup 1.0000x reference)
"""Deformable KPConv layer on 8 Trainium2 NeuronCores (Bass/Tile).

Strategy (data-parallel over the 16384 query points, 2048/core):
  - features are pre-cast to bf16 host-side and gathered per-edge from HBM
    via multi-index indirect DMA into an "edge-slot" layout
    [(4 queries x 32 neighbors) partitions, group, 128 feat].
  - support coords (+|s|^2) gathered per-edge in query-partition layout.
  - squared distances to (possibly deformed) kernel points are computed on
    DVE/GpSimd as  |s|^2 + |C|^2 - 2 s.C  with C = q + kp (+ offset),
    influence w' = relu(2 - d) on ScalarE (the 1/2 is folded into the
    conv weights host-side).
  - the neighbor contraction runs on TensorE as block-diagonal matmuls:
    w' is scattered into a zero-initialized block-diagonal SBUF tile by
    4 strided SBUF->SBUF DMAs, then  psum[f,(q,k)] = nf^T @ wblk.
  - the (k,f)->42 offset projection and (k,f)->256 output projection are
    PSUM-accumulated matmuls with the drained wf tiles as stationary
    operands, producing query-partition outputs directly.
"""

import os
import sys

sys.path.insert(0, "/opt/trn_rl_repo")

import numpy as np
import ml_dtypes

import concourse.bass as bass
import concourse.tile as tile
from concourse import bacc, mybir

N_Q = 16384
N_S = 16384
NN = 32
F_IN = 128
F_OUT = 256
K = 15
DIM = 3
OFF_DIM = DIM * (K - 1)  # 42
EXTENT = 2.0
N_CORES = 8
P = 128

BF16 = mybir.dt.bfloat16
F32 = mybir.dt.float32
I32 = mybir.dt.int32


def build_nc(qpc: int, reps: int = 1):
    """Build the single-core SPMD Bass program for qpc queries per core."""
    T = qpc // P  # query tiles per core
    NG = P // 4  # 32 groups of 4 queries per tile

    nc = bacc.Bacc("TRN2", target_bir_lowering=False)

    nfg_d = nc.dram_tensor("nfg", [T, P, NN, F_IN], BF16, kind="ExternalInput")
    spg_d = nc.dram_tensor("spg", [T, P, NN, 4], F32, kind="ExternalInput")
    qc_d = nc.dram_tensor("qc", [T, P, 4], F32, kind="ExternalInput")
    kprep_d = nc.dram_tensor("kprep", [P, K * DIM], F32, kind="ExternalInput")
    dwsb_d = nc.dram_tensor("dwsb", [P, K * OFF_DIM], BF16, kind="ExternalInput")
    wsb_d = nc.dram_tensor("wsb", [P, K * F_OUT], BF16, kind="ExternalInput")
    brep_d = nc.dram_tensor("brep", [P, OFF_DIM], F32, kind="ExternalInput")
    out_d = nc.dram_tensor("out", [qpc, F_OUT], F32, kind="ExternalOutput")

    NK = NN * K  # 480
    lvl = int(os.environ.get("KLVL", "5"))
    skips = set(os.environ.get("KSKIP", "").split(","))

    with tile.TileContext(nc) as tc:
        with (
            tc.tile_pool(name="const", bufs=1) as cpool,
            tc.tile_pool(name="persist", bufs=1) as ppool,
            tc.tile_pool(name="idx", bufs=2) as ipool,
            tc.tile_pool(name="nf", bufs=3) as nfpool,
            tc.tile_pool(name="sp", bufs=3) as sppool,
            tc.tile_pool(name="sq", bufs=3) as sqpool,
            tc.tile_pool(name="wd", bufs=3) as wdpool,
            tc.tile_pool(name="wf", bufs=2) as wfpool,
            tc.tile_pool(name="cc", bufs=2) as ccpool,
            tc.tile_pool(name="outp", bufs=2) as opool,
            tc.tile_pool(name="dram", bufs=4, space="DRAM") as drpool,
            tc.tile_pool(name="ps", bufs=3, space="PSUM") as pspool,
            tc.tile_pool(name="ps2", bufs=2, space="PSUM") as ps2pool,
        ):
            # --- constants, loaded once ---
            kprep = cpool.tile([P, K, DIM], F32, tag="kprep")
            nc.sync.dma_start(out=kprep[:], in_=kprep_d[:].rearrange("p (k d) -> p k d", d=DIM))
            dwsb = cpool.tile([P, K * OFF_DIM], BF16, tag="dwsb")
            nc.sync.dma_start(out=dwsb[:], in_=dwsb_d[:])
            wsb = cpool.tile([P, K * F_OUT], BF16, tag="wsb")
            nc.sync.dma_start(out=wsb[:], in_=wsb_d[:])
            brep = cpool.tile([P, OFF_DIM], F32, tag="brep")
            nc.sync.dma_start(out=brep[:], in_=brep_d[:])
            two_c = cpool.tile([P, 1], F32, tag="two")
            nc.vector.memset(two_c[:], 2.0)
            eps_c = cpool.tile([P, 1], F32, tag="eps")
            nc.vector.memset(eps_c[:], 1e-5)

            # persistent block-diagonal tiles (zeros off-diagonal, never touched
            # there again: the scatter DMAs only overwrite the diagonal blocks)
            wblks = []
            for i in range(6):
                wb = nc.alloc_sbuf_tensor(f"wblk{i}", [P, NG, 4 * K], BF16)
                nc.gpsimd.memset(wb.ap(), 0.0)
                wblks.append(wb)

            import itertools
            for _rep, t in itertools.product(range(reps), range(T)):
                # --- loads (pregathered on host per sharding strategy) ---
                qc_t = ipool.tile([P, 4], F32, tag="qc")
                nc.sync.dma_start(out=qc_t[:], in_=qc_d[t])
                nf = nfpool.tile([P, NN, F_IN], BF16, tag="nf")
                nc.sync.dma_start(out=nf[:], in_=nfg_d[t])
                sp = sppool.tile([P, NN, 4], F32, tag="sp")
                nc.sync.dma_start(out=sp[:], in_=spg_d[t])

                if lvl < 2:
                    out_sb0 = opool.tile([P, F_OUT], F32, tag="outsb")
                    nc.vector.memset(out_sb0[:], 0.0)
                    nc.vector.tensor_copy(out=out_sb0[:, 0:NN], in_=sp[:, :, 3])
                    nc.vector.tensor_copy(out=out_sb0[:, NN:NN+NN], in_=nf[:, 0, 0:NN])
                    nc.sync.dma_start(out=out_d[t * P : (t + 1) * P, :], in_=out_sb0[:])
                    continue
                # s' = -2*s, stored d-major [P, 4, NN]; row 3 = +|s|^2
                sp4t = sppool.tile([P, 4, NN], F32, tag="sp4t")
                nc.vector.tensor_scalar(
                    out=sp4t[:, 0:DIM, :],
                    in0=sp[:].transpose([0, 2, 1])[:, 0:DIM, :],
                    scalar1=-2.0,
                    scalar2=None,
                    op0=mybir.AluOpType.mult,
                )
                nc.vector.tensor_scalar(
                    out=sp4t[:, 3, :],
                    in0=sp[:, :, 3],
                    scalar1=1.0,
                    scalar2=None,
                    op0=mybir.AluOpType.mult,
                )

                # C0[q, k, d] = q_d + kp[k, d]
                c0 = ccpool.tile([P, K, DIM], F32, tag="c0")
                nc.vector.tensor_tensor(
                    out=c0[:],
                    in0=kprep[:],
                    in1=qc_t[:, 0:DIM].unsqueeze(1).broadcast_to([P, K, DIM]),
                    op=mybir.AluOpType.add,
                )

                wf_tiles = []
                c_cur = c0
                for stage in range(2):
                    def _emit_sq(c_cur):
                        csq = ccpool.tile([P, K, DIM], F32, tag="csq")
                        nc.vector.tensor_tensor(
                            out=csq[:], in0=c_cur[:], in1=c_cur[:], op=mybir.AluOpType.mult
                        )
                        cc = ccpool.tile([P, K], F32, tag="ccb")
                        nc.vector.tensor_reduce(
                            out=cc[:], in_=csq[:], axis=mybir.AxisListType.X,
                            op=mybir.AluOpType.add,
                        )
                        # sq[q, n, k] = (|s|^2 + |C|^2) + sum_d (-2 s_d) C_d
                        base = sqpool.tile([P, NN, K], F32, tag="base")
                        nc.gpsimd.tensor_tensor(
                            out=base[:],
                            in0=sp4t[:, 3, :].unsqueeze(2).broadcast_to([P, NN, K]),
                            in1=cc[:].unsqueeze(1).broadcast_to([P, NN, K]),
                            op=mybir.AluOpType.add,
                        )
                        tx = sqpool.tile([P, NN, K], F32, tag="tx")
                        ty = sqpool.tile([P, NN, K], F32, tag="ty")
                        nc.vector.tensor_tensor(
                            out=tx[:],
                            in0=sp4t[:, 0, :].unsqueeze(2).broadcast_to([P, NN, K]),
                            in1=c_cur[:, :, 0].unsqueeze(1).broadcast_to([P, NN, K]),
                            op=mybir.AluOpType.mult,
                        )
                        _eng1 = nc.vector if os.environ.get("KGPS") == "dve" else nc.gpsimd
                        _eng1.tensor_tensor(
                            out=ty[:],
                            in0=sp4t[:, 1, :].unsqueeze(2).broadcast_to([P, NN, K]),
                            in1=c_cur[:, :, 1].unsqueeze(1).broadcast_to([P, NN, K]),
                            op=mybir.AluOpType.mult,
                        )
                        nc.vector.tensor_tensor(
                            out=tx[:], in0=tx[:], in1=ty[:], op=mybir.AluOpType.add
                        )
                        ty2 = sqpool.tile([P, NN, K], F32, tag="ty2")
                        _eng1.tensor_tensor(
                            out=ty2[:],
                            in0=sp4t[:, 2, :].unsqueeze(2).broadcast_to([P, NN, K]),
                            in1=c_cur[:, :, 2].unsqueeze(1).broadcast_to([P, NN, K]),
                            op=mybir.AluOpType.mult,
                        )
                        nc.vector.tensor_tensor(
                            out=tx[:], in0=tx[:], in1=ty2[:], op=mybir.AluOpType.add
                        )
                        sqt = sqpool.tile([P, NN, K], F32, tag="sqt")
                        nc.vector.tensor_tensor(
                            out=sqt[:], in0=tx[:], in1=base[:], op=mybir.AluOpType.add
                        )

                        # influence: w' = relu(2 - sqrt(sq))
                        dts = wdpool.tile([P, NN, K], BF16, tag="dts")
                        nc.scalar.activation(
                            out=dts[:], in_=sqt[:],
                            func=mybir.ActivationFunctionType.Sqrt, bias=eps_c[:],
                        )
                        wdense = wdpool.tile([P, NN * K], BF16, tag="wdense")
                        nc.vector.tensor_scalar(
                            out=wdense[:],
                            in0=dts[:].rearrange("p n k -> p (n k)"),
                            scalar1=2.0,
                            scalar2=2.0,
                            op0=mybir.AluOpType.min,
                            op1=mybir.AluOpType.subtract,
                        )

                        return wdense

                    if "sq" in skips:
                        wdense = wdpool.tile([P, NN * K], BF16, tag="wdense")
                        nc.vector.memset(wdense[:], 0.5)
                    else:
                        wdense = _emit_sq(c_cur)

                    if lvl < 3:
                        if stage == 0:
                            out_sb0 = opool.tile([P, F_OUT], F32, tag="outsb")
                            nc.vector.memset(out_sb0[:], 0.0)
                            nc.vector.tensor_copy(out=out_sb0[:, 0:240], in_=wdense[:, 0:240])
                            nc.sync.dma_start(out=out_d[t * P : (t + 1) * P, :], in_=out_sb0[:])
                        break
                    # scatter into block-diagonal tile via DRAM bounce
                    # (partition remap q-layout -> edge-slot layout)
                    wblk = wblks[stage * 3 + (t % 3)].ap()
                    if "scatter" not in skips:
                        bounce = drpool.tile([P, NN * K], BF16, tag="bounce")
                        nc.sync.dma_start(out=bounce[:], in_=wdense[:])
                        wsrc = bounce[:].rearrange(
                            "(g qq) (n k) -> qq n g k", qq=4, k=K
                        )
                        for qq in range(4):
                            nc.sync.dma_start(
                                out=wblk[32 * qq : 32 * (qq + 1), :, K * qq : K * (qq + 1)],
                                in_=wsrc[qq],
                            )

                    if lvl < 4:
                        if stage == 0:
                            out_sb0 = opool.tile([P, F_OUT], F32, tag="outsb")
                            nc.vector.memset(out_sb0[:], 0.0)
                            nc.sync.dma_start(out=out_d[t * P : (t + 1) * P, :], in_=out_sb0[:])
                        break
                    # neighbor contraction:  psum[f, (qq,k)] += nf^T . wblk
                    wf_sb = wfpool.tile([P, K, P], BF16, tag=f"wf{stage}")
                    for b in ([] if "mm" in skips else range(4)):
                        psb = pspool.tile([P, 8 * 4 * K], F32, tag="psb")
                        for g8 in range(8):
                            g = b * 8 + g8
                            nc.tensor.matmul(
                                out=psb[:, g8 * 60 : (g8 + 1) * 60],
                                lhsT=nf[:, g, :],
                                rhs=wblk[:, g, :],
                                start=True,
                                stop=True,
                            )
                        # drain bank -> wf_sb[:, k, 32b:32b+32]  (k-major)
                        drain_src = psb[:].rearrange(
                            "p (g qq k) -> p k g qq", g=8, qq=4
                        )
                        drain_dst = (
                            wf_sb[:, :, 32 * b : 32 * (b + 1)]
                            .rearrange("p k (g qq) -> p k g qq", qq=4)
                        )
                        nc.vector.tensor_copy(out=drain_dst, in_=drain_src)
                    wf_tiles.append(wf_sb)

                    if lvl < 5:
                        if stage == 0:
                            out_sb0 = opool.tile([P, F_OUT], F32, tag="outsb")
                            nc.vector.tensor_copy(out=out_sb0[:, 0:128], in_=wf_sb[:, 0, :])
                            nc.vector.memset(out_sb0[:, 128:], 0.0)
                            nc.sync.dma_start(out=out_d[t * P : (t + 1) * P, :], in_=out_sb0[:])
                        break
                    if stage == 0:
                        # offset projection: feat0[q, o] = sum_k wf0_k^T . dw_k
                        psA = ps2pool.tile([P, OFF_DIM], F32, tag="psA")
                        for k in range(K):
                            nc.tensor.matmul(
                                out=psA[:],
                                lhsT=wf_sb[:, k, :],
                                rhs=dwsb[:, k * OFF_DIM : (k + 1) * OFF_DIM],
                                start=(k == 0),
                                stop=(k == K - 1),
                            )
                        off_sb = ccpool.tile([P, OFF_DIM], F32, tag="off")
                        nc.vector.tensor_tensor(
                            out=off_sb[:], in0=psA[:], in1=brep[:],
                            op=mybir.AluOpType.add,
                        )
                        # C1 = C0 + offsets (k=0 offset is zero)
                        c1 = ccpool.tile([P, K, DIM], F32, tag="c1")
                        nc.vector.tensor_copy(out=c1[:, 0, :], in_=c0[:, 0, :])
                        nc.vector.tensor_tensor(
                            out=c1[:, 1:K, :],
                            in0=c0[:, 1:K, :],
                            in1=off_sb[:].rearrange("p (k d) -> p k d", d=DIM),
                            op=mybir.AluOpType.add,
                        )
                        c_cur = c1

                if lvl < 5:
                    continue
                # output projection: out[q, o] = sum_k wf1_k^T . W_k
                psO = ps2pool.tile([P, F_OUT], F32, tag="psO")
                wf1 = wf_tiles[1]
                for k in range(K):
                    nc.tensor.matmul(
                        out=psO[:],
                        lhsT=wf1[:, k, :],
                        rhs=wsb[:, k * F_OUT : (k + 1) * F_OUT],
                        start=(k == 0),
                        stop=(k == K - 1),
                    )
                out_sb = opool.tile([P, F_OUT], F32, tag="outsb")
                nc.vector.tensor_copy(out=out_sb[:], in_=psO[:])
                nc.sync.dma_start(out=out_d[t * P : (t + 1) * P, :], in_=out_sb[:])

    nc.compile()
    return nc


def _prep_shared(support_points, features, K_points, weight, deformable_weight, bias):
    f16 = features.astype(ml_dtypes.bfloat16)
    sp4 = np.empty((N_S, 4), dtype=np.float32)
    sp4[:, :3] = support_points
    sp4[:, 3] = (support_points.astype(np.float64) ** 2).sum(1)
    kprep = np.broadcast_to(
        K_points.reshape(1, K * DIM), (P, K * DIM)
    ).astype(np.float32).copy()
    dwsb = (
        deformable_weight.transpose(1, 0, 2).reshape(F_IN, K * OFF_DIM) * -0.5
    ).astype(ml_dtypes.bfloat16)
    wsb = (
        weight.transpose(1, 0, 2).reshape(F_IN, K * F_OUT) * -0.5
    ).astype(ml_dtypes.bfloat16)
    brep = np.broadcast_to(bias.reshape(1, OFF_DIM), (P, OFF_DIM)).astype(
        np.float32
    ).copy()
    return f16, sp4, kprep, dwsb, wsb, brep


def _prep_core(query_points, neighbors, qpc, f16, sp4):
    """Shard-local tensors: pregathered neighbor features (edge-slot layout)
    and support coords (query layout), per the all-gather-neighbor-features
    sharding strategy."""
    T = qpc // P
    nbr = neighbors.astype(np.int64).reshape(T, P, NN)
    p = np.arange(P)
    g = np.arange(NN)
    # edge-slot permutation: ie[t, p, g] = nbr[t, 4g + p//32, p%32]
    ie = nbr[:, (4 * g[None, :] + p[:, None] // 32), (p[:, None] % 32)]
    nfg = np.asarray(f16)[ie]          # [T, P, NN, F_IN] bf16
    spg = sp4[nbr]                     # [T, P, NN, 4] f32
    qc = np.zeros((T, P, 4), dtype=np.float32)
    qc[:, :, :3] = query_points.reshape(T, P, DIM)
    return nfg, spg, qc


def kernel(query_points, support_points, neighbors, features, K_points,
           weight, deformable_weight, bias):
    from concourse.bass_utils import run_bass_kernel_spmd

    query_points = np.asarray(query_points, dtype=np.float32)
    support_points = np.asarray(support_points, dtype=np.float32)
    neighbors = np.asarray(neighbors)
    features = np.asarray(features, dtype=np.float32)
    K_points = np.asarray(K_points, dtype=np.float32)
    weight = np.asarray(weight, dtype=np.float32)
    deformable_weight = np.asarray(deformable_weight, dtype=np.float32)
    bias = np.asarray(bias, dtype=np.float32)

    qpc = N_Q // N_CORES
    f16, sp4, kprep, dwsb, wsb, brep = _prep_shared(
        support_points, features, K_points, weight, deformable_weight, bias)

    in_maps = []
    for c in range(N_CORES):
        sl = slice(c * qpc, (c + 1) * qpc)
        nfg, spg, qc = _prep_core(query_points[sl], neighbors[sl], qpc, f16, sp4)
        in_maps.append({
            "nfg": np.asarray(nfg), "spg": spg, "qc": qc,
            "kprep": kprep, "dwsb": np.asarray(dwsb), "wsb": np.asarray(wsb),
            "brep": brep,
        })

    nc = build_nc(qpc)
    res = run_bass_kernel_spmd(nc, in_maps, core_ids=list(range(N_CORES)))
    out = np.concatenate([res.results[c]["out"] for c in range(N_CORES)], axis=0)
    return out.astype(np.float32)



# revision 10
# speedup vs baseline: 1.0098x; 1.0098x over previous
"""Deformable KPConv layer on 8 Trainium2 NeuronCores (Bass/Tile).

Strategy (data-parallel over the 16384 query points, 2048/core):
  - features pre-cast to fp16 host-side and pregathered per-edge into an
    "edge-slot" layout [(4 queries x 32 neighbors) partitions, group, 128 feat].
  - support coords prepacked host-side as (-2x,-2y,-2z,|s|^2) fp16, d-major,
    in query-partition layout.
  - squared distances to (possibly deformed) kernel points computed fp16 on
    DVE/GpSimd as |s|^2 + |C|^2 - 2 s.C with C = q + kp (+ offset),
    influence w' = min(d,2) - 2 on ScalarE/DVE (sign+1/2 folded into the
    conv weights host-side).
  - neighbor contraction on TensorE as block-diagonal matmuls: w' is
    scattered into a zero-initialized block-diagonal SBUF tile by 4 direct
    SBUF->SBUF strided DMAs, then psum[f,(q,k)] = nf^T @ wblk.
  - PSUM drains are flat contiguous copies on the Scalar engine into
    wf[f, q, k]; the (k,f)->42 offset projection and (k,f)->256 output
    projection are PSUM-accumulated matmuls using strided wf[:, :, k] views
    as stationary operands, producing query-partition outputs directly.
"""

import os
import sys

sys.path.insert(0, "/opt/trn_rl_repo")

import numpy as np
import ml_dtypes

import concourse.bass as bass
import concourse.tile as tile
from concourse import bacc, mybir

N_Q = 16384
N_S = 16384
NN = 32
F_IN = 128
F_OUT = 256
K = 15
DIM = 3
OFF_DIM = DIM * (K - 1)  # 42
N_CORES = 8
P = 128

F16 = mybir.dt.float16
F32 = mybir.dt.float32

BOUNCE = os.environ.get("KBOUNCE", "1") == "1"  # DRAM bounce scatter (SBUF->SBUF
# partition-crossing DMA on both sides is rejected by the BIR verifier)


def build_nc(qpc: int):
    T = qpc // P  # query tiles per core
    NG = P // 4  # 32 groups of 4 queries per tile

    nc = bacc.Bacc("TRN2", target_bir_lowering=False)

    nfg_d = nc.dram_tensor("nfg", [T, P, NN, F_IN], F16, kind="ExternalInput")
    spg_d = nc.dram_tensor("spg", [T, P, 4, NN], F16, kind="ExternalInput")
    qc_d = nc.dram_tensor("qc", [T, P, 4], F32, kind="ExternalInput")
    kprep_d = nc.dram_tensor("kprep", [P, K * DIM], F32, kind="ExternalInput")
    dwsb_d = nc.dram_tensor("dwsb", [P, K * OFF_DIM], F16, kind="ExternalInput")
    wsb_d = nc.dram_tensor("wsb", [P, K * F_OUT], F16, kind="ExternalInput")
    brep_d = nc.dram_tensor("brep", [P, OFF_DIM], F32, kind="ExternalInput")
    out_d = nc.dram_tensor("out", [qpc, F_OUT], F32, kind="ExternalOutput")

    with tile.TileContext(nc) as tc:
        with (
            tc.tile_pool(name="const", bufs=1) as cpool,
            tc.tile_pool(name="idx", bufs=3) as ipool,
            tc.tile_pool(name="nf", bufs=3) as nfpool,
            tc.tile_pool(name="sp", bufs=3) as sppool,
            tc.tile_pool(name="sq", bufs=3) as sqpool,
            tc.tile_pool(name="wd", bufs=3) as wdpool,
            tc.tile_pool(name="wf", bufs=2) as wfpool,
            tc.tile_pool(name="cc", bufs=3) as ccpool,
            tc.tile_pool(name="outp", bufs=2) as opool,
            tc.tile_pool(name="dram", bufs=4, space="DRAM") as drpool,
            tc.tile_pool(name="ps", bufs=3, space="PSUM") as pspool,
            tc.tile_pool(name="ps2", bufs=2, space="PSUM") as ps2pool,
        ):
            # --- constants, loaded once ---
            kprep = cpool.tile([P, K, DIM], F32, tag="kprep")
            nc.sync.dma_start(out=kprep[:], in_=kprep_d[:].rearrange("p (k d) -> p k d", d=DIM))
            dwsb = cpool.tile([P, K * OFF_DIM], F16, tag="dwsb")
            nc.sync.dma_start(out=dwsb[:], in_=dwsb_d[:])
            wsb = cpool.tile([P, K * F_OUT], F16, tag="wsb")
            nc.sync.dma_start(out=wsb[:], in_=wsb_d[:])
            brep = cpool.tile([P, OFF_DIM], F32, tag="brep")
            nc.sync.dma_start(out=brep[:], in_=brep_d[:])
            # sqrt bias must cover fp16 rounding of sq (cancellation near
            # d=0 can make the computed sq slightly negative)
            eps_c = cpool.tile([P, 1], F32, tag="eps")
            nc.vector.memset(eps_c[:], 1e-2)

            # persistent block-diagonal tiles (zeros off-diagonal; only the
            # diagonal blocks are ever overwritten by the scatter DMAs)
            wblks = []
            for i in range(6):
                wb = nc.alloc_sbuf_tensor(f"wblk{i}", [P, NG, 4 * K], F16)
                nc.gpsimd.memset(wb.ap(), 0.0)
                wblks.append(wb)

            for t in range(T):
                # --- loads (pregathered on host per sharding strategy) ---
                qc_t = ipool.tile([P, 4], F32, tag="qc")
                nc.sync.dma_start(out=qc_t[:], in_=qc_d[t])
                nf = nfpool.tile([P, NN, F_IN], F16, tag="nf")
                nc.sync.dma_start(out=nf[:], in_=nfg_d[t])
                sp = sppool.tile([P, 4, NN], F16, tag="sp")
                nc.sync.dma_start(out=sp[:], in_=spg_d[t])

                # C0[q, k, d] = q_d + kp[k, d]
                c0 = ccpool.tile([P, K, DIM], F32, tag="c0")
                nc.vector.tensor_tensor(
                    out=c0[:],
                    in0=kprep[:],
                    in1=qc_t[:, 0:DIM].unsqueeze(1).broadcast_to([P, K, DIM]),
                    op=mybir.AluOpType.add,
                )

                wf_tiles = []
                c_cur = c0
                for stage in range(2):
                    # |C|^2 and fp16 casts of C
                    csq = ccpool.tile([P, K, DIM], F32, tag="csq")
                    nc.vector.tensor_tensor(
                        out=csq[:], in0=c_cur[:], in1=c_cur[:], op=mybir.AluOpType.mult
                    )
                    cc = ccpool.tile([P, K], F32, tag="ccb")
                    nc.vector.tensor_reduce(
                        out=cc[:], in_=csq[:], axis=mybir.AxisListType.X,
                        op=mybir.AluOpType.add,
                    )
                    ch = ccpool.tile([P, K, DIM], F16, tag="ch")
                    nc.vector.tensor_copy(out=ch[:], in_=c_cur[:])
                    cch = ccpool.tile([P, K], F16, tag="cch")
                    nc.vector.tensor_copy(out=cch[:], in_=cc[:])

                    # sq[q, n, k] = (|s|^2 + |C|^2) + sum_d (-2 s_d) C_d
                    base = sqpool.tile([P, NN, K], F16, tag="base")
                    nc.gpsimd.tensor_tensor(
                        out=base[:],
                        in0=sp[:, 3, :].unsqueeze(2).broadcast_to([P, NN, K]),
                        in1=cch[:].unsqueeze(1).broadcast_to([P, NN, K]),
                        op=mybir.AluOpType.add,
                    )
                    tx = sqpool.tile([P, NN, K], F16, tag="tx")
                    nc.vector.tensor_tensor(
                        out=tx[:],
                        in0=sp[:, 0, :].unsqueeze(2).broadcast_to([P, NN, K]),
                        in1=ch[:, :, 0].unsqueeze(1).broadcast_to([P, NN, K]),
                        op=mybir.AluOpType.mult,
                    )
                    ty = sqpool.tile([P, NN, K], F16, tag="ty")
                    nc.gpsimd.tensor_tensor(
                        out=ty[:],
                        in0=sp[:, 1, :].unsqueeze(2).broadcast_to([P, NN, K]),
                        in1=ch[:, :, 1].unsqueeze(1).broadcast_to([P, NN, K]),
                        op=mybir.AluOpType.mult,
                    )
                    tz = sqpool.tile([P, NN, K], F16, tag="tz")
                    nc.vector.tensor_tensor(
                        out=tz[:],
                        in0=sp[:, 2, :].unsqueeze(2).broadcast_to([P, NN, K]),
                        in1=ch[:, :, 2].unsqueeze(1).broadcast_to([P, NN, K]),
                        op=mybir.AluOpType.mult,
                    )
                    s1 = sqpool.tile([P, NN, K], F16, tag="s1")
                    nc.vector.tensor_tensor(
                        out=s1[:], in0=tx[:], in1=ty[:], op=mybir.AluOpType.add
                    )
                    s2 = sqpool.tile([P, NN, K], F16, tag="s2")
                    nc.gpsimd.tensor_tensor(
                        out=s2[:], in0=tz[:], in1=base[:], op=mybir.AluOpType.add
                    )
                    sqt = sqpool.tile([P, NN, K], F16, tag="sqt")
                    nc.vector.tensor_tensor(
                        out=sqt[:], in0=s1[:], in1=s2[:], op=mybir.AluOpType.add
                    )

                    # influence: w' = min(d,2) - 2  (sign folded into weights)
                    dts = wdpool.tile([P, NN, K], F16, tag="dts")
                    nc.scalar.activation(
                        out=dts[:], in_=sqt[:],
                        func=mybir.ActivationFunctionType.Sqrt, bias=eps_c[:],
                    )
                    wdense = wdpool.tile([P, NN * K], F16, tag="wdense")
                    nc.vector.tensor_scalar(
                        out=wdense[:],
                        in0=dts[:].rearrange("p n k -> p (n k)"),
                        scalar1=2.0,
                        scalar2=2.0,
                        op0=mybir.AluOpType.min,
                        op1=mybir.AluOpType.subtract,
                    )

                    # scatter into block-diagonal tile (q-layout -> edge-slot)
                    # via DRAM bounce (partition remap).
                    wblk = wblks[stage * 3 + (t % 3)].ap()
                    if BOUNCE:
                        bounce = drpool.tile([P, NN * K], F16, tag="bounce")
                        nc.sync.dma_start(out=bounce[:], in_=wdense[:])
                        wsrc = bounce[:].rearrange("(g qq) (n k) -> qq n g k", qq=4, k=K)
                    else:
                        wsrc = wdense[:].rearrange("(g qq) (n k) -> qq n g k", qq=4, k=K)
                    for qq in range(4):
                        nc.sync.dma_start(
                            out=wblk[32 * qq : 32 * (qq + 1), :, K * qq : K * (qq + 1)],
                            in_=wsrc[qq],
                        )

                    # neighbor contraction:  psum[f, (q-in-block, k)] = nf^T . wblk
                    wf_sb = wfpool.tile([P, P, K], F16, tag=f"wf{stage}")
                    for b in range(4):
                        psb = pspool.tile([P, 8 * 4 * K], F32, tag="psb")
                        for g8 in range(8):
                            g = b * 8 + g8
                            nc.tensor.matmul(
                                out=psb[:, g8 * 60 : (g8 + 1) * 60],
                                lhsT=nf[:, g, :],
                                rhs=wblk[:, g, :],
                                start=True,
                                stop=True,
                            )
                        # drain bank -> wf[f, 32b:32b+32, k]: flat contiguous copy
                        nc.scalar.activation(
                            out=wf_sb[:, 32 * b : 32 * (b + 1), :],
                            in_=psb[:].rearrange("p (q k) -> p q k", k=K),
                            func=mybir.ActivationFunctionType.Copy,
                        )
                    wf_tiles.append(wf_sb)

                    if stage == 0:
                        # offset projection: feat0[q, o] = sum_k wf0_k^T . dw_k
                        psA = ps2pool.tile([P, OFF_DIM], F32, tag="psA")
                        for k in range(K):
                            nc.tensor.matmul(
                                out=psA[:],
                                lhsT=wf_sb[:, :, k],
                                rhs=dwsb[:, k * OFF_DIM : (k + 1) * OFF_DIM],
                                start=(k == 0),
                                stop=(k == K - 1),
                            )
                        off_sb = ccpool.tile([P, OFF_DIM], F32, tag="off")
                        nc.vector.tensor_tensor(
                            out=off_sb[:], in0=psA[:], in1=brep[:],
                            op=mybir.AluOpType.add,
                        )
                        # C1 = C0 + offsets (k=0 offset is zero)
                        c1 = ccpool.tile([P, K, DIM], F32, tag="c1")
                        nc.vector.tensor_copy(out=c1[:, 0, :], in_=c0[:, 0, :])
                        nc.vector.tensor_tensor(
                            out=c1[:, 1:K, :],
                            in0=c0[:, 1:K, :],
                            in1=off_sb[:].rearrange("p (k d) -> p k d", d=DIM),
                            op=mybir.AluOpType.add,
                        )
                        c_cur = c1

                # output projection: out[q, o] = sum_k wf1_k^T . W_k
                psO = ps2pool.tile([P, F_OUT], F32, tag="psO")
                wf1 = wf_tiles[1]
                for k in range(K):
                    nc.tensor.matmul(
                        out=psO[:],
                        lhsT=wf1[:, :, k],
                        rhs=wsb[:, k * F_OUT : (k + 1) * F_OUT],
                        start=(k == 0),
                        stop=(k == K - 1),
                    )
                out_sb = opool.tile([P, F_OUT], F32, tag="outsb")
                nc.vector.tensor_copy(out=out_sb[:], in_=psO[:])
                nc.sync.dma_start(out=out_d[t * P : (t + 1) * P, :], in_=out_sb[:])

    nc.compile()
    return nc


def _prep_shared(support_points, features, K_points, weight, deformable_weight, bias):
    f16 = features.astype(ml_dtypes.float16) if False else features.astype(np.float16)
    # prepacked support rows: (-2x, -2y, -2z, |s|^2), d-major
    sp4n = np.empty((4, N_S), dtype=np.float32)
    sp4n[:3] = -2.0 * support_points.T
    sp4n[3] = (support_points.astype(np.float64) ** 2).sum(1)
    sp4n = sp4n.astype(np.float16)
    kprep = np.broadcast_to(
        K_points.reshape(1, K * DIM), (P, K * DIM)
    ).astype(np.float32).copy()
    dwsb = (
        deformable_weight.transpose(1, 0, 2).reshape(F_IN, K * OFF_DIM) * -0.5
    ).astype(np.float16)
    wsb = (
        weight.transpose(1, 0, 2).reshape(F_IN, K * F_OUT) * -0.5
    ).astype(np.float16)
    brep = np.broadcast_to(bias.reshape(1, OFF_DIM), (P, OFF_DIM)).astype(
        np.float32
    ).copy()
    return f16, sp4n, kprep, dwsb, wsb, brep


def _prep_core(query_points, neighbors, qpc, f16, sp4n):
    """Shard-local tensors: pregathered neighbor features (edge-slot layout)
    and prepacked support coords (query layout, d-major)."""
    T = qpc // P
    nbr = neighbors.astype(np.int64).reshape(T, P, NN)
    p = np.arange(P)
    g = np.arange(NN)
    # edge-slot permutation: ie[t, p, g] = nbr[t, 4g + p//32, p%32]
    ie = nbr[:, (4 * g[None, :] + p[:, None] // 32), (p[:, None] % 32)]
    nfg = f16[ie]                          # [T, P, NN, F_IN] fp16
    spg = sp4n[:, nbr].transpose(1, 2, 0, 3).copy()  # [T, P, 4, NN] fp16
    qc = np.zeros((T, P, 4), dtype=np.float32)
    qc[:, :, :3] = query_points.reshape(T, P, DIM)
    return nfg, spg, qc


def kernel(query_points, support_points, neighbors, features, K_points,
           weight, deformable_weight, bias):
    from concourse.bass_utils import run_bass_kernel_spmd

    query_points = np.asarray(query_points, dtype=np.float32)
    support_points = np.asarray(support_points, dtype=np.float32)
    neighbors = np.asarray(neighbors)
    features = np.asarray(features, dtype=np.float32)
    K_points = np.asarray(K_points, dtype=np.float32)
    weight = np.asarray(weight, dtype=np.float32)
    deformable_weight = np.asarray(deformable_weight, dtype=np.float32)
    bias = np.asarray(bias, dtype=np.float32)

    qpc = N_Q // N_CORES
    f16, sp4n, kprep, dwsb, wsb, brep = _prep_shared(
        support_points, features, K_points, weight, deformable_weight, bias)

    in_maps = []
    for c in range(N_CORES):
        sl = slice(c * qpc, (c + 1) * qpc)
        nfg, spg, qc = _prep_core(query_points[sl], neighbors[sl], qpc, f16, sp4n)
        in_maps.append({
            "nfg": nfg, "spg": spg, "qc": qc,
            "kprep": kprep, "dwsb": dwsb, "wsb": wsb,
            "brep": brep,
        })

    nc = build_nc(qpc)
    res = run_bass_kernel_spmd(nc, in_maps, core_ids=list(range(N_CORES)))
    out = np.concatenate([res.results[c]["out"] for c in range(N_CORES)], axis=0)
    return out.astype(np.float32)


# revision 12
# speedup vs baseline: 1.5529x; 1.5379x over previous
"""Deformable KPConv layer on 8 Trainium2 NeuronCores (Bass/Tile).

Strategy (data-parallel over the 16384 query points, 2048/core):
  - features pre-cast to fp16 host-side and pregathered per-edge into an
    "edge-slot" layout [(4 queries x 32 neighbors) partitions, group, 128 feat].
  - relative neighbor coords (s - q) prepacked fp16 d-major host-side in
    query-partition layout; squared distances to the (possibly deformed)
    kernel points are computed difference-first (|rel - C|^2 with
    C = kp + offset) on DVE/GpSimd in fp16 -- no cancellation, sq >= 0 by
    construction. Influence w' = min(d,2) - 2 (sign+1/2 folded into the
    conv weights host-side).
  - neighbor contraction on TensorE as block-diagonal matmuls: w' is
    scattered into a zero-initialized block-diagonal SBUF tile via a DRAM
    bounce (partition remap), then psum[f,(q,k)] = nf^T @ wblk.
  - PSUM drains are flat contiguous copies on the Scalar engine into
    wf[f, q, k]; the (k,f)->42 offset projection and (k,f)->256 output
    projection are PSUM-accumulated matmuls with strided wf[:, :, k] views
    as stationary operands, producing query-partition outputs directly.
  - emission is software-pipelined two deep: stage 1 of tile t-1 is
    interleaved after stage 0 of tile t, so in-order engine queues always
    have ready work behind a cross-stage dependency wait.
"""

import os
import sys

sys.path.insert(0, "/opt/trn_rl_repo")

import numpy as np

import concourse.bass as bass
import concourse.tile as tile
from concourse import bacc, mybir

N_Q = 16384
N_S = 16384
NN = 32
F_IN = 128
F_OUT = 256
K = 15
DIM = 3
OFF_DIM = DIM * (K - 1)  # 42
N_CORES = 8
P = 128

F16 = mybir.dt.float16
F32 = mybir.dt.float32


def build_nc(qpc: int):
    T = qpc // P  # query tiles per core
    NG = P // 4  # 32 groups of 4 queries per tile

    nc = bacc.Bacc("TRN2", target_bir_lowering=False)

    nfg_d = nc.dram_tensor("nfg", [T, P, NN, F_IN], F16, kind="ExternalInput")
    relg_d = nc.dram_tensor("relg", [T, P, DIM, NN], F16, kind="ExternalInput")
    kprep_d = nc.dram_tensor("kprep", [P, K * DIM], F32, kind="ExternalInput")
    dwsb_d = nc.dram_tensor("dwsb", [P, K * OFF_DIM], F16, kind="ExternalInput")
    wsb_d = nc.dram_tensor("wsb", [P, K * F_OUT], F16, kind="ExternalInput")
    brep_d = nc.dram_tensor("brep", [P, OFF_DIM], F32, kind="ExternalInput")
    out_d = nc.dram_tensor("out", [qpc, F_OUT], F32, kind="ExternalOutput")

    with tile.TileContext(nc) as tc:
        with (
            tc.tile_pool(name="const", bufs=1) as cpool,
            tc.tile_pool(name="nf", bufs=3) as nfpool,
            tc.tile_pool(name="rl", bufs=3) as rlpool,
            tc.tile_pool(name="sq", bufs=3) as sqpool,
            tc.tile_pool(name="wd", bufs=3) as wdpool,
            tc.tile_pool(name="wf", bufs=3) as wfpool,
            tc.tile_pool(name="cc", bufs=3) as ccpool,
            tc.tile_pool(name="outp", bufs=2) as opool,
            tc.tile_pool(name="dram", bufs=6, space="DRAM") as drpool,
            tc.tile_pool(name="ps", bufs=3, space="PSUM") as pspool,
            tc.tile_pool(name="ps2", bufs=2, space="PSUM") as ps2pool,
        ):
            # --- constants, loaded once ---
            kprep = cpool.tile([P, K, DIM], F32, tag="kprep")
            nc.sync.dma_start(out=kprep[:], in_=kprep_d[:].rearrange("p (k d) -> p k d", d=DIM))
            dwsb = cpool.tile([P, K * OFF_DIM], F16, tag="dwsb")
            nc.sync.dma_start(out=dwsb[:], in_=dwsb_d[:])
            wsb = cpool.tile([P, K * F_OUT], F16, tag="wsb")
            nc.sync.dma_start(out=wsb[:], in_=wsb_d[:])
            brep = cpool.tile([P, OFF_DIM], F32, tag="brep")
            nc.sync.dma_start(out=brep[:], in_=brep_d[:])
            eps_c = cpool.tile([P, 1], F32, tag="eps")
            nc.vector.memset(eps_c[:], 1e-6)
            # fp16 kernel points for the stage-0 (rigid) distances
            kh0 = cpool.tile([P, K, DIM], F16, tag="kh0")
            nc.vector.tensor_copy(out=kh0[:], in_=kprep[:])

            # persistent block-diagonal tiles (zeros off-diagonal; only the
            # diagonal blocks are ever overwritten by the scatter DMAs)
            wblks = []
            for i in range(6):
                wb = nc.alloc_sbuf_tensor(f"wblk{i}", [P, NG, 4 * K], F16)
                nc.gpsimd.memset(wb.ap(), 0.0)
                wblks.append(wb)

            # per-tile state carried between pipeline phases
            state = {}

            def emit_load(t):
                nf = nfpool.tile([P, NN, F_IN], F16, tag="nf")
                nc.sync.dma_start(out=nf[:], in_=nfg_d[t])
                rl = rlpool.tile([P, DIM, NN], F16, tag="rl")
                nc.sync.dma_start(out=rl[:], in_=relg_d[t])
                state[t] = {"nf": nf, "rl": rl}

            def emit_dists(t, stage, ch):
                """fp16 |rel - C|^2 -> w' = min(d,2)-2, scattered into wblk."""
                rl = state[t]["rl"]

                def bc_rel(d):
                    return rl[:, d, :].unsqueeze(2).broadcast_to([P, NN, K])

                def bc_c(d):
                    return ch[:, :, d].unsqueeze(1).broadcast_to([P, NN, K])

                dx = sqpool.tile([P, NN, K], F16, tag="dx")
                nc.vector.tensor_tensor(
                    out=dx[:], in0=bc_rel(0), in1=bc_c(0), op=mybir.AluOpType.subtract
                )
                dy = sqpool.tile([P, NN, K], F16, tag="dy")
                nc.gpsimd.tensor_tensor(
                    out=dy[:], in0=bc_rel(1), in1=bc_c(1), op=mybir.AluOpType.subtract
                )
                dz = sqpool.tile([P, NN, K], F16, tag="dz")
                nc.vector.tensor_tensor(
                    out=dz[:], in0=bc_rel(2), in1=bc_c(2), op=mybir.AluOpType.subtract
                )
                mx = sqpool.tile([P, NN, K], F16, tag="mx")
                nc.gpsimd.tensor_tensor(
                    out=mx[:], in0=dx[:], in1=dx[:], op=mybir.AluOpType.mult
                )
                my = sqpool.tile([P, NN, K], F16, tag="my")
                nc.vector.tensor_tensor(
                    out=my[:], in0=dy[:], in1=dy[:], op=mybir.AluOpType.mult
                )
                mz = sqpool.tile([P, NN, K], F16, tag="mz")
                nc.vector.tensor_tensor(
                    out=mz[:], in0=dz[:], in1=dz[:], op=mybir.AluOpType.mult
                )
                a1 = sqpool.tile([P, NN, K], F16, tag="a1")
                nc.gpsimd.tensor_tensor(
                    out=a1[:], in0=mx[:], in1=my[:], op=mybir.AluOpType.add
                )
                sqt = sqpool.tile([P, NN, K], F16, tag="sqt")
                nc.vector.tensor_tensor(
                    out=sqt[:], in0=a1[:], in1=mz[:], op=mybir.AluOpType.add
                )

                # influence: w' = min(d,2) - 2  (sign folded into weights)
                dts = wdpool.tile([P, NN, K], F16, tag="dts")
                nc.scalar.activation(
                    out=dts[:], in_=sqt[:],
                    func=mybir.ActivationFunctionType.Sqrt, bias=eps_c[:],
                )
                wdense = wdpool.tile([P, NN * K], F16, tag="wdense")
                nc.vector.tensor_scalar(
                    out=wdense[:],
                    in0=dts[:].rearrange("p n k -> p (n k)"),
                    scalar1=2.0,
                    scalar2=2.0,
                    op0=mybir.AluOpType.min,
                    op1=mybir.AluOpType.subtract,
                )

                # scatter into block-diagonal tile (q-layout -> edge-slot)
                # via DRAM bounce (partition remap)
                wblk = wblks[stage * 3 + (t % 3)].ap()
                bounce = drpool.tile([P, NN * K], F16, tag="bounce")
                nc.scalar.dma_start(out=bounce[:], in_=wdense[:])
                wsrc = bounce[:].rearrange("(g qq) (n k) -> qq n g k", qq=4, k=K)
                for qq in range(4):
                    nc.sync.dma_start(
                        out=wblk[32 * qq : 32 * (qq + 1), :, K * qq : K * (qq + 1)],
                        in_=wsrc[qq],
                    )
                return wblk

            def emit_contract(t, stage, wblk):
                """psum[f, (q-in-block, k)] = nf^T . wblk, drained to
                wf[f, q, k] (flat copies on the Scalar engine)."""
                nf = state[t]["nf"]
                wf_sb = wfpool.tile([P, P, K], F16, tag=f"wf{stage}")
                for b in range(4):
                    psb = pspool.tile([P, 8 * 4 * K], F32, tag="psb")
                    for g8 in range(8):
                        g = b * 8 + g8
                        nc.tensor.matmul(
                            out=psb[:, g8 * 60 : (g8 + 1) * 60],
                            lhsT=nf[:, g, :],
                            rhs=wblk[:, g, :],
                            start=True,
                            stop=True,
                        )
                    nc.scalar.activation(
                        out=wf_sb[:, 32 * b : 32 * (b + 1), :],
                        in_=psb[:].rearrange("p (q k) -> p q k", k=K),
                        func=mybir.ActivationFunctionType.Copy,
                    )
                return wf_sb

            def emit_stage0(t):
                emit_load(t)
                wblk = emit_dists(t, 0, kh0)
                wf0 = emit_contract(t, 0, wblk)
                # offset projection: feat0[q, o] = sum_k wf0_k^T . dw_k
                psA = ps2pool.tile([P, OFF_DIM], F32, tag="psA")
                for k in range(K):
                    nc.tensor.matmul(
                        out=psA[:],
                        lhsT=wf0[:, :, k],
                        rhs=dwsb[:, k * OFF_DIM : (k + 1) * OFF_DIM],
                        start=(k == 0),
                        stop=(k == K - 1),
                    )
                state[t]["psA"] = psA

            def emit_stage1(t):
                # C1 = kp + offsets (k=0 offset is zero); offsets stay f32
                psA = state[t].pop("psA")
                off_sb = ccpool.tile([P, OFF_DIM], F32, tag="off")
                nc.vector.tensor_tensor(
                    out=off_sb[:], in0=psA[:], in1=brep[:], op=mybir.AluOpType.add
                )
                c1 = ccpool.tile([P, K, DIM], F32, tag="c1")
                nc.vector.tensor_copy(out=c1[:, 0, :], in_=kprep[:, 0, :])
                nc.vector.tensor_tensor(
                    out=c1[:, 1:K, :],
                    in0=kprep[:, 1:K, :],
                    in1=off_sb[:].rearrange("p (k d) -> p k d", d=DIM),
                    op=mybir.AluOpType.add,
                )
                ch1 = ccpool.tile([P, K, DIM], F16, tag="ch1")
                nc.vector.tensor_copy(out=ch1[:], in_=c1[:])

                wblk = emit_dists(t, 1, ch1)
                wf1 = emit_contract(t, 1, wblk)
                # output projection: out[q, o] = sum_k wf1_k^T . W_k
                psO = ps2pool.tile([P, F_OUT], F32, tag="psO")
                for k in range(K):
                    nc.tensor.matmul(
                        out=psO[:],
                        lhsT=wf1[:, :, k],
                        rhs=wsb[:, k * F_OUT : (k + 1) * F_OUT],
                        start=(k == 0),
                        stop=(k == K - 1),
                    )
                out_sb = opool.tile([P, F_OUT], F32, tag="outsb")
                nc.scalar.activation(
                    out=out_sb[:], in_=psO[:],
                    func=mybir.ActivationFunctionType.Copy,
                )
                nc.sync.dma_start(out=out_d[t * P : (t + 1) * P, :], in_=out_sb[:])
                del state[t]

            # two-deep software pipeline over tiles
            for step in range(T + 1):
                if step < T:
                    emit_stage0(step)
                if step >= 1:
                    emit_stage1(step - 1)

    nc.compile()
    return nc


def _prep_shared(support_points, features, K_points, weight, deformable_weight, bias):
    f16 = features.astype(np.float16)
    spT = support_points.T.astype(np.float32)  # [3, N_S]
    kprep = np.broadcast_to(
        K_points.reshape(1, K * DIM), (P, K * DIM)
    ).astype(np.float32).copy()
    dwsb = (
        deformable_weight.transpose(1, 0, 2).reshape(F_IN, K * OFF_DIM) * -0.5
    ).astype(np.float16)
    wsb = (
        weight.transpose(1, 0, 2).reshape(F_IN, K * F_OUT) * -0.5
    ).astype(np.float16)
    brep = np.broadcast_to(bias.reshape(1, OFF_DIM), (P, OFF_DIM)).astype(
        np.float32
    ).copy()
    return f16, spT, kprep, dwsb, wsb, brep


def _prep_core(query_points, neighbors, qpc, f16, spT):
    """Shard-local tensors: pregathered neighbor features (edge-slot layout)
    and relative neighbor coords (query layout, d-major)."""
    T = qpc // P
    nbr = neighbors.astype(np.int64).reshape(T, P, NN)
    p = np.arange(P)
    g = np.arange(NN)
    # edge-slot permutation: ie[t, p, g] = nbr[t, 4g + p//32, p%32]
    ie = nbr[:, (4 * g[None, :] + p[:, None] // 32), (p[:, None] % 32)]
    nfg = f16[ie]                          # [T, P, NN, F_IN] fp16
    # rel[t, p, d, n] = s[nbr[t,p,n], d] - q[t,p,d]
    qp = query_points.reshape(T, P, DIM)
    relg = (spT[:, nbr].transpose(1, 2, 0, 3) - qp[:, :, :, None]).astype(
        np.float16
    )  # [T, P, 3, NN]
    return nfg, relg


def build_in_maps(query_points, support_points, neighbors, features, K_points,
                  weight, deformable_weight, bias):
    qpc = N_Q // N_CORES
    f16, spT, kprep, dwsb, wsb, brep = _prep_shared(
        support_points, features, K_points, weight, deformable_weight, bias)
    in_maps = []
    for c in range(N_CORES):
        sl = slice(c * qpc, (c + 1) * qpc)
        nfg, relg = _prep_core(query_points[sl], np.asarray(neighbors)[sl],
                               qpc, f16, spT)
        in_maps.append({
            "nfg": nfg, "relg": relg,
            "kprep": kprep, "dwsb": dwsb, "wsb": wsb, "brep": brep,
        })
    return qpc, in_maps


def kernel(query_points, support_points, neighbors, features, K_points,
           weight, deformable_weight, bias):
    from concourse.bass_utils import run_bass_kernel_spmd

    query_points = np.asarray(query_points, dtype=np.float32)
    support_points = np.asarray(support_points, dtype=np.float32)
    neighbors = np.asarray(neighbors)
    features = np.asarray(features, dtype=np.float32)
    K_points = np.asarray(K_points, dtype=np.float32)
    weight = np.asarray(weight, dtype=np.float32)
    deformable_weight = np.asarray(deformable_weight, dtype=np.float32)
    bias = np.asarray(bias, dtype=np.float32)

    qpc, in_maps = build_in_maps(
        query_points, support_points, neighbors, features, K_points,
        weight, deformable_weight, bias)
    nc = build_nc(qpc)
    res = run_bass_kernel_spmd(nc, in_maps, core_ids=list(range(N_CORES)))
    out = np.concatenate([res.results[c]["out"] for c in range(N_CORES)], axis=0)
    return out.astype(np.float32)


# revision 15
# speedup vs baseline: 1.7469x; 1.1249x over previous
"""Deformable KPConv layer on 8 Trainium2 NeuronCores (Bass/Tile).

Strategy (data-parallel over the 16384 query points, 2048/core):
  - features pre-cast to fp16 host-side and pregathered per-edge into an
    "edge-slot" layout [(4 queries x 32 neighbors) partitions, group, 128 feat].
  - relative neighbor coords (s - q) prepacked fp16 d-major host-side in
    query-partition layout; squared distances to the (possibly deformed)
    kernel points are computed difference-first (|rel - C|^2 with
    C = kp + offset) on DVE/GpSimd in fp16 -- no cancellation, sq >= 0 by
    construction. Influence w' = min(d,2) - 2 (sign+1/2 folded into the
    conv weights host-side).
  - neighbor contraction on TensorE as block-diagonal matmuls: w' is
    scattered into a zero-initialized block-diagonal SBUF tile via a DRAM
    bounce (partition remap), then psum[f,(q,k)] = nf^T @ wblk.
  - PSUM drains are flat contiguous copies on the Scalar engine into
    wf[f, q, k]; the (k,f)->42 offset projection and (k,f)->256 output
    projection are PSUM-accumulated matmuls with strided wf[:, :, k] views
    as stationary operands, producing query-partition outputs directly.
  - emission is software-pipelined two deep: stage 1 of tile t-1 is
    interleaved after stage 0 of tile t, so in-order engine queues always
    have ready work behind a cross-stage dependency wait.
"""

import os
import sys

sys.path.insert(0, "/opt/trn_rl_repo")

import numpy as np

import concourse.bass as bass
import concourse.tile as tile
from concourse import bacc, mybir

N_Q = 16384
N_S = 16384
NN = 32
F_IN = 128
F_OUT = 256
K = 15
DIM = 3
OFF_DIM = DIM * (K - 1)  # 42
N_CORES = 8
P = 128

F16 = mybir.dt.float16
F32 = mybir.dt.float32


def build_nc(qpc: int):
    T = qpc // P  # query tiles per core
    NG = P // 4  # 32 groups of 4 queries per tile

    nc = bacc.Bacc("TRN2", target_bir_lowering=False)

    nfg_d = nc.dram_tensor("nfg", [T, P, NN, F_IN], F16, kind="ExternalInput")
    relg_d = nc.dram_tensor("relg", [T, P, DIM, NN], F16, kind="ExternalInput")
    kprep_d = nc.dram_tensor("kprep", [P, K * DIM], F32, kind="ExternalInput")
    dwsb_d = nc.dram_tensor("dwsb", [P, K * OFF_DIM], F16, kind="ExternalInput")
    wsb_d = nc.dram_tensor("wsb", [P, K * F_OUT], F16, kind="ExternalInput")
    brep_d = nc.dram_tensor("brep", [P, OFF_DIM], F32, kind="ExternalInput")
    out_d = nc.dram_tensor("out", [qpc, F_OUT], F32, kind="ExternalOutput")

    with tile.TileContext(nc) as tc:
        with (
            tc.tile_pool(name="const", bufs=1) as cpool,
            tc.tile_pool(name="nf", bufs=5) as nfpool,
            tc.tile_pool(name="rl", bufs=4) as rlpool,
            tc.tile_pool(name="sq", bufs=3) as sqpool,
            tc.tile_pool(name="wd", bufs=3) as wdpool,
            tc.tile_pool(name="wf", bufs=3) as wfpool,
            tc.tile_pool(name="cc", bufs=3) as ccpool,
            tc.tile_pool(name="outp", bufs=2) as opool,
            tc.tile_pool(name="dram", bufs=6, space="DRAM") as drpool,
            tc.tile_pool(name="ps", bufs=3, space="PSUM") as pspool,
            tc.tile_pool(name="ps2", bufs=2, space="PSUM") as ps2pool,
        ):
            # --- constants, loaded once ---
            kprep = cpool.tile([P, K, DIM], F32, tag="kprep")
            nc.sync.dma_start(out=kprep[:], in_=kprep_d[:].rearrange("p (k d) -> p k d", d=DIM))
            dwsb = cpool.tile([P, K * OFF_DIM], F16, tag="dwsb")
            nc.sync.dma_start(out=dwsb[:], in_=dwsb_d[:])
            wsb = cpool.tile([P, K * F_OUT], F16, tag="wsb")
            nc.sync.dma_start(out=wsb[:], in_=wsb_d[:])
            brep = cpool.tile([P, OFF_DIM], F32, tag="brep")
            nc.sync.dma_start(out=brep[:], in_=brep_d[:])
            eps_c = cpool.tile([P, 1], F32, tag="eps")
            nc.vector.memset(eps_c[:], 1e-6)
            # fp16 kernel points for the stage-0 (rigid) distances
            kh0 = cpool.tile([P, K, DIM], F16, tag="kh0")
            nc.vector.tensor_copy(out=kh0[:], in_=kprep[:])

            # persistent block-diagonal tiles (zeros off-diagonal; only the
            # diagonal blocks are ever overwritten by the scatter DMAs)
            wblks = []
            for i in range(6):
                wb = nc.alloc_sbuf_tensor(f"wblk{i}", [P, NG, 4 * K], F16)
                nc.gpsimd.memset(wb.ap(), 0.0)
                wblks.append(wb)

            # per-tile state carried between pipeline phases
            state = {}

            def emit_load(t):
                nf = nfpool.tile([P, NN, F_IN], F16, tag="nf")
                nc.sync.dma_start(out=nf[:], in_=nfg_d[t])
                rl = rlpool.tile([P, DIM, NN], F16, tag="rl")
                nc.sync.dma_start(out=rl[:], in_=relg_d[t])
                state[t] = {"nf": nf, "rl": rl}

            def emit_dists(t, stage, ch):
                """fp16 |rel - C|^2 -> w' = min(d,2)-2, scattered into wblk."""
                rl = state[t]["rl"]

                def bc_rel(d):
                    return rl[:, d, :].unsqueeze(2).broadcast_to([P, NN, K])

                def bc_c(d):
                    return ch[:, :, d].unsqueeze(1).broadcast_to([P, NN, K])

                dx = sqpool.tile([P, NN, K], F16, tag="dx")
                nc.vector.tensor_tensor(
                    out=dx[:], in0=bc_rel(0), in1=bc_c(0), op=mybir.AluOpType.subtract
                )
                dy = sqpool.tile([P, NN, K], F16, tag="dy")
                nc.gpsimd.tensor_tensor(
                    out=dy[:], in0=bc_rel(1), in1=bc_c(1), op=mybir.AluOpType.subtract
                )
                dz = sqpool.tile([P, NN, K], F16, tag="dz")
                nc.vector.tensor_tensor(
                    out=dz[:], in0=bc_rel(2), in1=bc_c(2), op=mybir.AluOpType.subtract
                )
                mx = sqpool.tile([P, NN, K], F16, tag="mx")
                nc.gpsimd.tensor_tensor(
                    out=mx[:], in0=dx[:], in1=dx[:], op=mybir.AluOpType.mult
                )
                my = sqpool.tile([P, NN, K], F16, tag="my")
                nc.vector.tensor_tensor(
                    out=my[:], in0=dy[:], in1=dy[:], op=mybir.AluOpType.mult
                )
                mz = sqpool.tile([P, NN, K], F16, tag="mz")
                nc.vector.tensor_tensor(
                    out=mz[:], in0=dz[:], in1=dz[:], op=mybir.AluOpType.mult
                )
                a1 = sqpool.tile([P, NN, K], F16, tag="a1")
                nc.gpsimd.tensor_tensor(
                    out=a1[:], in0=mx[:], in1=my[:], op=mybir.AluOpType.add
                )
                sqt = sqpool.tile([P, NN, K], F16, tag="sqt")
                nc.vector.tensor_tensor(
                    out=sqt[:], in0=a1[:], in1=mz[:], op=mybir.AluOpType.add
                )

                # influence: w' = min(d,2) - 2  (sign folded into weights)
                dts = wdpool.tile([P, NN, K], F16, tag="dts")
                nc.scalar.activation(
                    out=dts[:], in_=sqt[:],
                    func=mybir.ActivationFunctionType.Sqrt, bias=eps_c[:],
                )
                wdense = wdpool.tile([P, NN * K], F16, tag="wdense")
                nc.vector.tensor_scalar(
                    out=wdense[:],
                    in0=dts[:].rearrange("p n k -> p (n k)"),
                    scalar1=2.0,
                    scalar2=2.0,
                    op0=mybir.AluOpType.min,
                    op1=mybir.AluOpType.subtract,
                )

                # scatter into block-diagonal tile (q-layout -> edge-slot)
                # via DRAM bounce (partition remap)
                wblk = wblks[stage * 3 + (t % 3)].ap()
                bounce = drpool.tile([P, NN * K], F16, tag="bounce")
                nc.scalar.dma_start(out=bounce[:], in_=wdense[:])
                wsrc = bounce[:].rearrange("(g qq) (n k) -> qq n g k", qq=4, k=K)
                for qq in range(4):
                    nc.sync.dma_start(
                        out=wblk[32 * qq : 32 * (qq + 1), :, K * qq : K * (qq + 1)],
                        in_=wsrc[qq],
                    )
                return wblk

            def emit_contract(t, stage, wblk):
                """psum[f, (q-in-block, k)] = nf^T . wblk, drained to
                wf[f, q, k] (flat copies on the Scalar engine)."""
                nf = state[t]["nf"]
                wf_sb = wfpool.tile([P, P, K], F16, tag=f"wf{stage}")
                for b in range(4):
                    psb = pspool.tile([P, 8 * 4 * K], F32, tag="psb")
                    for g8 in range(8):
                        g = b * 8 + g8
                        nc.tensor.matmul(
                            out=psb[:, g8 * 60 : (g8 + 1) * 60],
                            lhsT=nf[:, g, :],
                            rhs=wblk[:, g, :],
                            start=True,
                            stop=True,
                        )
                    nc.scalar.activation(
                        out=wf_sb[:, 32 * b : 32 * (b + 1), :],
                        in_=psb[:].rearrange("p (q k) -> p q k", k=K),
                        func=mybir.ActivationFunctionType.Copy,
                    )
                return wf_sb

            def phase_A(t):
                emit_load(t)
                state[t]["wblk0"] = emit_dists(t, 0, kh0)

            def phase_B(t):
                wf0 = emit_contract(t, 0, state[t].pop("wblk0"))
                # offset projection: feat0[q, o] = sum_k wf0_k^T . dw_k
                psA = ps2pool.tile([P, OFF_DIM], F32, tag="psA")
                for k in range(K):
                    nc.tensor.matmul(
                        out=psA[:],
                        lhsT=wf0[:, :, k],
                        rhs=dwsb[:, k * OFF_DIM : (k + 1) * OFF_DIM],
                        start=(k == 0),
                        stop=(k == K - 1),
                    )
                state[t]["psA"] = psA

            def phase_C(t):
                # C1 = kp + offsets (k=0 offset is zero); offsets stay f32
                psA = state[t].pop("psA")
                off_sb = ccpool.tile([P, OFF_DIM], F32, tag="off")
                nc.vector.tensor_tensor(
                    out=off_sb[:], in0=psA[:], in1=brep[:], op=mybir.AluOpType.add
                )
                c1 = ccpool.tile([P, K, DIM], F32, tag="c1")
                nc.vector.tensor_copy(out=c1[:, 0, :], in_=kprep[:, 0, :])
                nc.vector.tensor_tensor(
                    out=c1[:, 1:K, :],
                    in0=kprep[:, 1:K, :],
                    in1=off_sb[:].rearrange("p (k d) -> p k d", d=DIM),
                    op=mybir.AluOpType.add,
                )
                ch1 = ccpool.tile([P, K, DIM], F16, tag="ch1")
                nc.vector.tensor_copy(out=ch1[:], in_=c1[:])
                state[t]["wblk1"] = emit_dists(t, 1, ch1)

            def phase_D(t):
                wf1 = emit_contract(t, 1, state[t].pop("wblk1"))
                # output projection: out[q, o] = sum_k wf1_k^T . W_k
                psO = ps2pool.tile([P, F_OUT], F32, tag="psO")
                for k in range(K):
                    nc.tensor.matmul(
                        out=psO[:],
                        lhsT=wf1[:, :, k],
                        rhs=wsb[:, k * F_OUT : (k + 1) * F_OUT],
                        start=(k == 0),
                        stop=(k == K - 1),
                    )
                out_sb = opool.tile([P, F_OUT], F32, tag="outsb")
                nc.scalar.activation(
                    out=out_sb[:], in_=psO[:],
                    func=mybir.ActivationFunctionType.Copy,
                )
                nc.scalar.dma_start(
                    out=out_d[t * P : (t + 1) * P, :], in_=out_sb[:]
                )
                del state[t]

            # four-deep software pipeline over tiles; oldest phase emitted
            # first so fresh semaphore waits sit at queue tails
            for step in range(T + 3):
                if 3 <= step < T + 3:
                    phase_D(step - 3)
                if 2 <= step < T + 2:
                    phase_C(step - 2)
                if 1 <= step < T + 1:
                    phase_B(step - 1)
                if step < T:
                    phase_A(step)

    nc.compile()
    return nc


def _prep_shared(support_points, features, K_points, weight, deformable_weight, bias):
    f16 = features.astype(np.float16)
    spT = support_points.T.astype(np.float32)  # [3, N_S]
    kprep = np.broadcast_to(
        K_points.reshape(1, K * DIM), (P, K * DIM)
    ).astype(np.float32).copy()
    dwsb = (
        deformable_weight.transpose(1, 0, 2).reshape(F_IN, K * OFF_DIM) * -0.5
    ).astype(np.float16)
    wsb = (
        weight.transpose(1, 0, 2).reshape(F_IN, K * F_OUT) * -0.5
    ).astype(np.float16)
    brep = np.broadcast_to(bias.reshape(1, OFF_DIM), (P, OFF_DIM)).astype(
        np.float32
    ).copy()
    return f16, spT, kprep, dwsb, wsb, brep


def _prep_core(query_points, neighbors, qpc, f16, spT):
    """Shard-local tensors: pregathered neighbor features (edge-slot layout)
    and relative neighbor coords (query layout, d-major)."""
    T = qpc // P
    nbr = neighbors.astype(np.int64).reshape(T, P, NN)
    p = np.arange(P)
    g = np.arange(NN)
    # edge-slot permutation: ie[t, p, g] = nbr[t, 4g + p//32, p%32]
    ie = nbr[:, (4 * g[None, :] + p[:, None] // 32), (p[:, None] % 32)]
    nfg = f16[ie]                          # [T, P, NN, F_IN] fp16
    # rel[t, p, d, n] = s[nbr[t,p,n], d] - q[t,p,d]
    qp = query_points.reshape(T, P, DIM)
    relg = (spT[:, nbr].transpose(1, 2, 0, 3) - qp[:, :, :, None]).astype(
        np.float16
    )  # [T, P, 3, NN]
    return nfg, relg


def build_in_maps(query_points, support_points, neighbors, features, K_points,
                  weight, deformable_weight, bias):
    qpc = N_Q // N_CORES
    f16, spT, kprep, dwsb, wsb, brep = _prep_shared(
        support_points, features, K_points, weight, deformable_weight, bias)
    in_maps = []
    for c in range(N_CORES):
        sl = slice(c * qpc, (c + 1) * qpc)
        nfg, relg = _prep_core(query_points[sl], np.asarray(neighbors)[sl],
                               qpc, f16, spT)
        in_maps.append({
            "nfg": nfg, "relg": relg,
            "kprep": kprep, "dwsb": dwsb, "wsb": wsb, "brep": brep,
        })
    return qpc, in_maps


def kernel(query_points, support_points, neighbors, features, K_points,
           weight, deformable_weight, bias):
    from concourse.bass_utils import run_bass_kernel_spmd

    query_points = np.asarray(query_points, dtype=np.float32)
    support_points = np.asarray(support_points, dtype=np.float32)
    neighbors = np.asarray(neighbors)
    features = np.asarray(features, dtype=np.float32)
    K_points = np.asarray(K_points, dtype=np.float32)
    weight = np.asarray(weight, dtype=np.float32)
    deformable_weight = np.asarray(deformable_weight, dtype=np.float32)
    bias = np.asarray(bias, dtype=np.float32)

    qpc, in_maps = build_in_maps(
        query_points, support_points, neighbors, features, K_points,
        weight, deformable_weight, bias)
    nc = build_nc(qpc)
    res = run_bass_kernel_spmd(nc, in_maps, core_ids=list(range(N_CORES)))
    out = np.concatenate([res.results[c]["out"] for c in range(N_CORES)], axis=0)
    return out.astype(np.float32)
